# revision 17
# baseline (speedup 1.0000x reference)
"""Conv4d (kernel 3^4, circular, grouped-over-time) on 8 TRN2 NeuronCores.

Math: res[b,co,t] = sum_g conv3d_valid(pad_wrap1(x[b,:,s=t-1+g]), W[g]) + bias,
with s circular over the 16 time slices.

Device scheme (per core = one (batch, 8-time-slice) shard):
  - outputs processed in pairs (t, t+1); PSUM partitions = (t-sel u, c_out)
  - contraction K = (input-slice-sel j, c_in) over pair-tiles of two
    consecutive padded slices stacked on partitions
  - per (kd,kh,kw) tap and output pair: 2 matmuls, K=128 M=128 N=512 bf16:
      L-block: slices (t-1, t),  g = j - u      (g=-1 entry zeroed)
      H-block: slices (t+1, t+2), g = j - u + 2 (g=3 entry zeroed)
  - rhs = 3-level-AP window into the padded 18^3 slice cube; kw=1 windows
    would start 2B-misaligned (halfword) which slows the PE rhs fetch, so a
    w-shifted duplicate of each cube serves kw=1 at offset 0 instead
  - 2 output d-planes per matmul -> 8 PSUM banks cover the 16^3 output
  - bias added during PSUM->SBUF evacuation (DVE tensor_scalar_add)
"""
import numpy as np

B, C, S, KW = 4, 64, 16, 3
SP = S + 2           # padded spatial extent
CUBE = SP * SP * SP  # 5832 padded elements per channel
DSTR = SP * SP       # elements per d-plane (324)
NCORES = 8
TSH = S * B // NCORES  # 8 output time slices per core
NWARM = 6

_PROGRAM = None


def _build_program():
    import concourse.bacc as bacc
    import concourse.mybir as mybir
    import concourse.tile as tile

    nc = bacc.Bacc("TRN2", target_bir_lowering=False, debug=False,
                   num_devices=NCORES)
    bf16 = mybir.dt.bfloat16
    f32 = mybir.dt.float32

    xs_d = nc.dram_tensor("xs", [5, 128, CUBE], bf16, kind="ExternalInput").ap()
    xw_d = nc.dram_tensor("xw", [5, 128, CUBE], bf16, kind="ExternalInput").ap()
    wl_d = nc.dram_tensor("wl", [128, 27 * 128], bf16, kind="ExternalInput").ap()
    wh_d = nc.dram_tensor("wh", [128, 27 * 128], bf16, kind="ExternalInput").ap()
    bias_d = nc.dram_tensor("bias2", [128, 1], f32, kind="ExternalInput").ap()
    y_d = nc.dram_tensor("y", [TSH, C, S * S * S], f32, kind="ExternalOutput").ap()

    with tile.TileContext(nc) as tc:
        with (
            tc.tile_pool(name="xp", bufs=1) as xpool,
            tc.tile_pool(name="wp", bufs=1) as wpool,
            tc.tile_pool(name="st", bufs=2) as spool,
            tc.tile_pool(name="ps", bufs=8, space="PSUM") as pspool,
        ):
            wlt = wpool.tile([128, 27 * 128], bf16)
            wht = wpool.tile([128, 27 * 128], bf16)
            bias_t = wpool.tile([128, 1], f32)
            dummy = wpool.tile([128, 512], bf16)
            xts = [xpool.tile([128, CUBE], bf16, name=f"xt{k}")
                   for k in range(5)]
            xss = [xpool.tile([128, CUBE], bf16, name=f"xs{k}")
                   for k in range(5)]

            # small PE warm-up: start the HAM clock ramp while DMAs land
            nc.any.memset(dummy, 0)
            warm = pspool.tile([128, 512], f32, name="bank")
            for i in range(NWARM):
                nc.tensor.matmul(warm[:], dummy[:, 0:128], dummy[:],
                                 start=(i == 0), stop=(i == NWARM - 1))

            # d-plane ranges per piece; chunk c needs planes [2c, 2c+4)
            PIECES = [(0, 4), (4, 11), (11, 18)]
            wpiece = 7 * 128  # 7 taps of weights

            def wdma_l(p):
                lo, hi = p * wpiece, min((p + 1) * wpiece, 27 * 128)
                nc.sync.dma_start(wlt[:, lo:hi], wl_d[:, lo:hi])

            def wdma_h(p):
                lo, hi = p * wpiece, min((p + 1) * wpiece, 27 * 128)
                nc.scalar.dma_start(wht[:, lo:hi], wh_d[:, lo:hi])

            def xdma(q, t, d, k, p):
                lo, hi = PIECES[p]
                q.dma_start(t[k][:, lo * DSTR:hi * DSTR],
                            d[k][:, lo * DSTR:hi * DSTR])

            # All supply on the fast hwdge queues (SP + Activation); the
            # gpsimd software-DGE path is ~2us/transfer and was gating the
            # first matmuls when it carried the weights.  L-weights + xt0
            # on sync, H-weights + xt1 on scalar, interleaved so tap i's
            # needs land in stream order; gpsimd gets only the bias.
            wdma_l(0)
            wdma_h(0)
            nc.gpsimd.dma_start(bias_t[:], bias_d)
            xdma(nc.sync, xts, xs_d, 0, 0)
            xdma(nc.scalar, xts, xs_d, 1, 0)
            xdma(nc.sync, xss, xw_d, 0, 0)
            xdma(nc.scalar, xss, xw_d, 1, 0)
            wdma_l(1)
            wdma_h(1)
            xdma(nc.sync, xts, xs_d, 0, 1)
            xdma(nc.scalar, xts, xs_d, 1, 1)
            xdma(nc.sync, xss, xw_d, 0, 1)
            xdma(nc.scalar, xss, xw_d, 1, 1)
            wdma_l(2)
            wdma_h(2)
            xdma(nc.sync, xts, xs_d, 0, 2)
            xdma(nc.scalar, xts, xs_d, 1, 2)
            xdma(nc.sync, xss, xw_d, 0, 2)
            xdma(nc.scalar, xss, xw_d, 1, 2)
            wdma_l(3)
            wdma_h(3)
            for k in (2, 3, 4):
                for p in range(3):
                    xdma(nc.sync, xts, xs_d, k, p)
                    xdma(nc.scalar, xss, xw_d, k, p)

            xvs = [xt.rearrange("p (d h w) -> p d h w", d=SP, h=SP, w=SP)
                   for xt in xts]
            xsv = [xt.rearrange("p (d h w) -> p d h w", d=SP, h=SP, w=SP)
                   for xt in xss]

            TAPS = [(kd, kh, kw)
                    for kd in range(KW) for kh in range(KW) for kw in range(KW)]

            for u in range(TSH // 2):  # output pair
                stage = spool.tile([128, S * S * S], f32, name="stage")
                for c in range(8):  # 2 output d-planes per chunk
                    bank = pspool.tile([128, 512], f32, name="bank")
                    nmm = 0
                    for kd, kh, kw in TAPS:
                        i = (kd * KW + kh) * KW + kw
                        vs = xsv if kw == 1 else xvs
                        wb = 0 if kw == 1 else kw
                        for wt, xv in ((wlt, vs[u]), (wht, vs[u + 1])):
                            rhs = xv[:, 2 * c + kd:2 * c + kd + 2,
                                     kh:kh + S, wb:wb + S]
                            nc.tensor.matmul(
                                bank[:],
                                wt[:, i * 128:(i + 1) * 128],
                                rhs,
                                start=(nmm == 0), stop=(nmm == 53),
                            )
                            nmm += 1
                    nc.vector.tensor_scalar_add(
                        stage[:, c * 512:(c + 1) * 512], bank[:], bias_t[:]
                    )
                    yq = nc.sync if c % 2 == 0 else nc.scalar
                    yq.dma_start(
                        y_d[2 * u][:, c * 512:(c + 1) * 512],
                        stage[0:C, c * 512:(c + 1) * 512],
                    )
                    yq.dma_start(
                        y_d[2 * u + 1][:, c * 512:(c + 1) * 512],
                        stage[C:128, c * 512:(c + 1) * 512],
                    )

    nc.compile()
    return nc


def _host_prep(x, weight, bias):
    """Build per-core input maps."""
    import ml_dtypes

    bf16 = ml_dtypes.bfloat16
    # padded slices: xp[b, s] = wrap-pad1 of x[b,:,s] -> (C, 18,18,18)
    xpad = np.pad(x, ((0, 0), (0, 0), (0, 0), (1, 1), (1, 1), (1, 1)),
                  mode="wrap").astype(bf16)  # (B, C, S, 18,18,18)
    # w-shifted copy: xsh[..., w] = xpad[..., w+1] (only w<17 is ever read)
    xsh = np.concatenate(
        [xpad[..., 1:], np.zeros_like(xpad[..., 0:1])], axis=-1)

    # weight block-banded lhsT tiles: [128=(j,ci), 27*128=(tap,(u,co))]
    wl = np.zeros((128, 27, 128), dtype=np.float32)
    wh = np.zeros((128, 27, 128), dtype=np.float32)
    for kd in range(KW):
        for kh in range(KW):
            for kw in range(KW):
                i = (kd * KW + kh) * KW + kw
                for j in range(2):
                    for u in range(2):
                        gl = j - u
                        if 0 <= gl < KW:
                            wl[j * C:(j + 1) * C, i, u * C:(u + 1) * C] = \
                                weight[gl, :, :, kd, kh, kw].T
                        gh = j - u + 2
                        if 0 <= gh < KW:
                            wh[j * C:(j + 1) * C, i, u * C:(u + 1) * C] = \
                                weight[gh, :, :, kd, kh, kw].T
    wl = wl.reshape(128, 27 * 128).astype(bf16)
    wh = wh.reshape(128, 27 * 128).astype(bf16)
    bias2 = np.concatenate([bias, bias]).astype(np.float32).reshape(128, 1)

    in_maps = []
    for core in range(NCORES):
        b = core // 2
        t0 = TSH * (core % 2)
        xs = np.empty((5, 128, CUBE), dtype=bf16)
        xw = np.empty((5, 128, CUBE), dtype=bf16)
        for k in range(5):
            sa = (t0 - 1 + 2 * k) % S
            sb = (t0 + 2 * k) % S
            xs[k, 0:C] = xpad[b, :, sa].reshape(C, CUBE)
            xs[k, C:128] = xpad[b, :, sb].reshape(C, CUBE)
            xw[k, 0:C] = xsh[b, :, sa].reshape(C, CUBE)
            xw[k, C:128] = xsh[b, :, sb].reshape(C, CUBE)
        in_maps.append({"xs": xs, "xw": xw, "wl": wl, "wh": wh,
                        "bias2": bias2})
    return in_maps


LAST_RESULTS = None


def kernel(x, weight, bias, _trace=False):
    global _PROGRAM, LAST_RESULTS
    from concourse import bass_utils

    x = np.asarray(x, dtype=np.float32)
    weight = np.asarray(weight, dtype=np.float32)
    bias = np.asarray(bias, dtype=np.float32)

    if _PROGRAM is None:
        _PROGRAM = _build_program()
    nc = _PROGRAM

    in_maps = _host_prep(x, weight, bias)
    res = bass_utils.run_bass_kernel_spmd(
        nc, in_maps, core_ids=list(range(NCORES)), trace=_trace
    )
    LAST_RESULTS = res

    out = np.empty((B, C, S, S, S, S), dtype=np.float32)
    for core in range(NCORES):
        b = core // 2
        t0 = TSH * (core % 2)
        y = res.results[core]["y"]  # (TSH, C, 4096)
        out[b, :, t0:t0 + TSH] = y.transpose(1, 0, 2).reshape(C, TSH, S, S, S)
    return out


# revision 19
# speedup vs baseline: 1.0099x; 1.0099x over previous
"""Conv4d (kernel 3^4, circular, grouped-over-time) on 8 TRN2 NeuronCores.

Math: res[b,co,t] = sum_g conv3d_valid(pad_wrap1(x[b,:,s=t-1+g]), W[g]) + bias,
with s circular over the 16 time slices.

Device scheme (per core = one (batch, 8-time-slice) shard):
  - outputs processed in pairs (t, t+1); PSUM partitions = (t-sel u, c_out)
  - contraction K = (input-slice-sel j, c_in) over pair-tiles of two
    consecutive padded slices stacked on partitions
  - per (kd,kh,kw) tap and output pair: 2 matmuls, K=128 M=128 N=512 bf16:
      L-block: slices (t-1, t),  g = j - u      (g=-1 entry zeroed)
      H-block: slices (t+1, t+2), g = j - u + 2 (g=3 entry zeroed)
  - rhs = 3-level-AP window into the padded 18^3 slice cube; kw=1 windows
    would start 2B-misaligned (halfword) which slows the PE rhs fetch, so a
    w-shifted duplicate of each cube serves kw=1 at offset 0 instead
  - 2 output d-planes per matmul -> 8 PSUM banks cover the 16^3 output
  - bias added during PSUM->SBUF evacuation (DVE tensor_scalar_add)
"""
import numpy as np

B, C, S, KW = 4, 64, 16, 3
SP = S + 2           # padded spatial extent
CUBE = SP * SP * SP  # 5832 padded elements per channel
DSTR = SP * SP       # elements per d-plane (324)
NCORES = 8
TSH = S * B // NCORES  # 8 output time slices per core
NWARM = 6

_PROGRAM = None


def _build_program():
    import concourse.bacc as bacc
    import concourse.mybir as mybir
    import concourse.tile as tile

    nc = bacc.Bacc("TRN2", target_bir_lowering=False, debug=False,
                   num_devices=NCORES)
    bf16 = mybir.dt.bfloat16
    f32 = mybir.dt.float32

    xs_d = nc.dram_tensor("xs", [5, 128, CUBE], bf16, kind="ExternalInput").ap()
    xw_d = nc.dram_tensor("xw", [5, 128, CUBE], bf16, kind="ExternalInput").ap()
    wl_d = nc.dram_tensor("wl", [128, 27 * 128], bf16, kind="ExternalInput").ap()
    wh_d = nc.dram_tensor("wh", [128, 27 * 128], bf16, kind="ExternalInput").ap()
    bias_d = nc.dram_tensor("bias2", [128, 1], f32, kind="ExternalInput").ap()
    y_d = nc.dram_tensor("y", [TSH, C, S * S * S], f32, kind="ExternalOutput").ap()

    with tile.TileContext(nc) as tc:
        with (
            tc.tile_pool(name="xp", bufs=1) as xpool,
            tc.tile_pool(name="wp", bufs=1) as wpool,
            tc.tile_pool(name="st", bufs=2) as spool,
            tc.tile_pool(name="ps", bufs=8, space="PSUM") as pspool,
        ):
            wlt = wpool.tile([128, 27 * 128], bf16)
            wht = wpool.tile([128, 27 * 128], bf16)
            bias_t = wpool.tile([128, 1], f32)
            dummy = wpool.tile([128, 512], bf16)
            xts = [xpool.tile([128, CUBE], bf16, name=f"xt{k}")
                   for k in range(5)]
            xss = [xpool.tile([128, CUBE], bf16, name=f"xs{k}")
                   for k in range(5)]

            # small PE warm-up: start the HAM clock ramp while DMAs land
            nc.any.memset(dummy, 0)
            warm = pspool.tile([128, 512], f32, name="bank")
            for i in range(NWARM):
                nc.tensor.matmul(warm[:], dummy[:, 0:128], dummy[:],
                                 start=(i == 0), stop=(i == NWARM - 1))

            # piece layout: fine plane-pairs for the startup-critical
            # tiles (k=0,1), coarse for the rest
            PIECES = [(0, 2), (2, 4), (4, 11), (11, 18)]
            CPIECES = [(0, 4), (4, 11), (11, 18)]

            def wdma(q, wt_t, w_d, lo_t, hi_t):
                q.dma_start(wt_t[:, lo_t * 128:hi_t * 128],
                            w_d[:, lo_t * 128:hi_t * 128])

            def xdma(q, t, d, k, p, pieces=PIECES):
                lo, hi = pieces[p]
                q.dma_start(t[k][:, lo * DSTR:hi * DSTR],
                            d[k][:, lo * DSTR:hi * DSTR])

            # Three-lane supply scheduled to chunk 0's consumption order
            # (taps kd-major, 2 matmuls per tap, ~0.25us each early):
            #   sync (hwdge):   wl taps 0-8, xt0/xss0 plane pieces, wl 21-26
            #   scalar (hwdge): wh taps 0-8, xt1/xss1 plane pieces, wh 21-26
            #   gpsimd (sw, ~4us/piece): bias + both roles of taps 9-20,
            #     which are not needed until ~18-25us into the run
            nc.gpsimd.dma_start(bias_t[:], bias_d)
            wdma(nc.gpsimd, wlt, wl_d, 9, 14)
            wdma(nc.gpsimd, wht, wh_d, 9, 14)
            wdma(nc.gpsimd, wlt, wl_d, 14, 21)
            wdma(nc.gpsimd, wht, wh_d, 14, 21)
            wdma(nc.sync, wlt, wl_d, 0, 9)
            wdma(nc.scalar, wht, wh_d, 0, 9)
            xdma(nc.sync, xts, xs_d, 0, 0)
            xdma(nc.scalar, xts, xs_d, 1, 0)
            xdma(nc.sync, xss, xw_d, 0, 0)
            xdma(nc.scalar, xss, xw_d, 1, 0)
            xdma(nc.sync, xts, xs_d, 0, 1)
            xdma(nc.scalar, xts, xs_d, 1, 1)
            xdma(nc.sync, xss, xw_d, 0, 1)
            xdma(nc.scalar, xss, xw_d, 1, 1)
            wdma(nc.sync, wlt, wl_d, 21, 27)
            wdma(nc.scalar, wht, wh_d, 21, 27)
            for p in (2, 3):
                xdma(nc.sync, xts, xs_d, 0, p)
                xdma(nc.scalar, xts, xs_d, 1, p)
                xdma(nc.sync, xss, xw_d, 0, p)
                xdma(nc.scalar, xss, xw_d, 1, p)
            for k in (2, 3, 4):
                for p in range(3):
                    xdma(nc.sync, xts, xs_d, k, p, CPIECES)
                    xdma(nc.scalar, xss, xw_d, k, p, CPIECES)

            xvs = [xt.rearrange("p (d h w) -> p d h w", d=SP, h=SP, w=SP)
                   for xt in xts]
            xsv = [xt.rearrange("p (d h w) -> p d h w", d=SP, h=SP, w=SP)
                   for xt in xss]

            TAPS = [(kd, kh, kw)
                    for kd in range(KW) for kh in range(KW) for kw in range(KW)]

            for u in range(TSH // 2):  # output pair
                stage = spool.tile([128, S * S * S], f32, name="stage")
                for c in range(8):  # 2 output d-planes per chunk
                    bank = pspool.tile([128, 512], f32, name="bank")
                    nmm = 0
                    for kd, kh, kw in TAPS:
                        i = (kd * KW + kh) * KW + kw
                        vs = xsv if kw == 1 else xvs
                        wb = 0 if kw == 1 else kw
                        for wt, xv in ((wlt, vs[u]), (wht, vs[u + 1])):
                            rhs = xv[:, 2 * c + kd:2 * c + kd + 2,
                                     kh:kh + S, wb:wb + S]
                            nc.tensor.matmul(
                                bank[:],
                                wt[:, i * 128:(i + 1) * 128],
                                rhs,
                                start=(nmm == 0), stop=(nmm == 53),
                            )
                            nmm += 1
                    nc.vector.tensor_scalar_add(
                        stage[:, c * 512:(c + 1) * 512], bank[:], bias_t[:]
                    )
                    # the two time-slice writes go on separate hwdge lanes
                    # so the final chunk's output drains in parallel
                    nc.sync.dma_start(
                        y_d[2 * u][:, c * 512:(c + 1) * 512],
                        stage[0:C, c * 512:(c + 1) * 512],
                    )
                    nc.scalar.dma_start(
                        y_d[2 * u + 1][:, c * 512:(c + 1) * 512],
                        stage[C:128, c * 512:(c + 1) * 512],
                    )

    nc.compile()
    return nc


def _host_prep(x, weight, bias):
    """Build per-core input maps."""
    import ml_dtypes

    bf16 = ml_dtypes.bfloat16
    # padded slices: xp[b, s] = wrap-pad1 of x[b,:,s] -> (C, 18,18,18)
    xpad = np.pad(x, ((0, 0), (0, 0), (0, 0), (1, 1), (1, 1), (1, 1)),
                  mode="wrap").astype(bf16)  # (B, C, S, 18,18,18)
    # w-shifted copy: xsh[..., w] = xpad[..., w+1] (only w<17 is ever read)
    xsh = np.concatenate(
        [xpad[..., 1:], np.zeros_like(xpad[..., 0:1])], axis=-1)

    # weight block-banded lhsT tiles: [128=(j,ci), 27*128=(tap,(u,co))]
    wl = np.zeros((128, 27, 128), dtype=np.float32)
    wh = np.zeros((128, 27, 128), dtype=np.float32)
    for kd in range(KW):
        for kh in range(KW):
            for kw in range(KW):
                i = (kd * KW + kh) * KW + kw
                for j in range(2):
                    for u in range(2):
                        gl = j - u
                        if 0 <= gl < KW:
                            wl[j * C:(j + 1) * C, i, u * C:(u + 1) * C] = \
                                weight[gl, :, :, kd, kh, kw].T
                        gh = j - u + 2
                        if 0 <= gh < KW:
                            wh[j * C:(j + 1) * C, i, u * C:(u + 1) * C] = \
                                weight[gh, :, :, kd, kh, kw].T
    wl = wl.reshape(128, 27 * 128).astype(bf16)
    wh = wh.reshape(128, 27 * 128).astype(bf16)
    bias2 = np.concatenate([bias, bias]).astype(np.float32).reshape(128, 1)

    in_maps = []
    for core in range(NCORES):
        b = core // 2
        t0 = TSH * (core % 2)
        xs = np.empty((5, 128, CUBE), dtype=bf16)
        xw = np.empty((5, 128, CUBE), dtype=bf16)
        for k in range(5):
            sa = (t0 - 1 + 2 * k) % S
            sb = (t0 + 2 * k) % S
            xs[k, 0:C] = xpad[b, :, sa].reshape(C, CUBE)
            xs[k, C:128] = xpad[b, :, sb].reshape(C, CUBE)
            xw[k, 0:C] = xsh[b, :, sa].reshape(C, CUBE)
            xw[k, C:128] = xsh[b, :, sb].reshape(C, CUBE)
        in_maps.append({"xs": xs, "xw": xw, "wl": wl, "wh": wh,
                        "bias2": bias2})
    return in_maps


LAST_RESULTS = None


def kernel(x, weight, bias, _trace=False):
    global _PROGRAM, LAST_RESULTS
    from concourse import bass_utils

    x = np.asarray(x, dtype=np.float32)
    weight = np.asarray(weight, dtype=np.float32)
    bias = np.asarray(bias, dtype=np.float32)

    if _PROGRAM is None:
        _PROGRAM = _build_program()
    nc = _PROGRAM

    in_maps = _host_prep(x, weight, bias)
    res = bass_utils.run_bass_kernel_spmd(
        nc, in_maps, core_ids=list(range(NCORES)), trace=_trace
    )
    LAST_RESULTS = res

    out = np.empty((B, C, S, S, S, S), dtype=np.float32)
    for core in range(NCORES):
        b = core // 2
        t0 = TSH * (core % 2)
        y = res.results[core]["y"]  # (TSH, C, 4096)
        out[b, :, t0:t0 + TSH] = y.transpose(1, 0, 2).reshape(C, TSH, S, S, S)
    return out


# revision 22
# speedup vs baseline: 1.0116x; 1.0017x over previous
"""Conv4d (kernel 3^4, circular, grouped-over-time) on 8 TRN2 NeuronCores.

Math: res[b,co,t] = sum_g conv3d_valid(pad_wrap1(x[b,:,s=t-1+g]), W[g]) + bias,
with s circular over the 16 time slices.

Device scheme (per core = one (batch, 8-time-slice) shard):
  - outputs processed in pairs (t, t+1); PSUM partitions = (t-sel u, c_out)
  - contraction K = (input-slice-sel j, c_in) over pair-tiles of two
    consecutive padded slices stacked on partitions
  - per (kd,kh,kw) tap and output pair: 2 matmuls, K=128 M=128 N=512 bf16:
      L-block: slices (t-1, t),  g = j - u      (g=-1 entry zeroed)
      H-block: slices (t+1, t+2), g = j - u + 2 (g=3 entry zeroed)
  - rhs = 3-level-AP window into the padded 18^3 slice cube; kw=1 windows
    would start 2B-misaligned (halfword) which slows the PE rhs fetch, so a
    w-shifted duplicate of each cube serves kw=1 at offset 0 instead
  - 2 output d-planes per matmul -> 8 PSUM banks cover the 16^3 output
  - bias added during PSUM->SBUF evacuation (DVE tensor_scalar_add)
"""
import numpy as np

B, C, S, KW = 4, 64, 16, 3
SP = S + 2           # padded spatial extent
CUBE = SP * SP * SP  # 5832 padded elements per channel
DSTR = SP * SP       # elements per d-plane (324)
NCORES = 8
TSH = S * B // NCORES  # 8 output time slices per core
NWARM = 4

_PROGRAM = None


def _build_program():
    import concourse.bacc as bacc
    import concourse.mybir as mybir
    import concourse.tile as tile

    nc = bacc.Bacc("TRN2", target_bir_lowering=False, debug=False,
                   num_devices=NCORES)
    bf16 = mybir.dt.bfloat16
    f32 = mybir.dt.float32

    xs_d = nc.dram_tensor("xs", [5, 128, CUBE], bf16, kind="ExternalInput").ap()
    xw_d = nc.dram_tensor("xw", [5, 128, CUBE], bf16, kind="ExternalInput").ap()
    wl_d = nc.dram_tensor("wl", [128, 27 * 128], bf16, kind="ExternalInput").ap()
    wh_d = nc.dram_tensor("wh", [128, 27 * 128], bf16, kind="ExternalInput").ap()
    bias_d = nc.dram_tensor("bias2", [128, 1], f32, kind="ExternalInput").ap()
    y_d = nc.dram_tensor("y", [TSH, C, S * S * S], f32, kind="ExternalOutput").ap()

    with tile.TileContext(nc) as tc:
        with (
            tc.tile_pool(name="xp", bufs=1) as xpool,
            tc.tile_pool(name="wp", bufs=1) as wpool,
            tc.tile_pool(name="st", bufs=2) as spool,
            tc.tile_pool(name="ps", bufs=8, space="PSUM") as pspool,
        ):
            wlt = wpool.tile([128, 27 * 128], bf16)
            wht = wpool.tile([128, 27 * 128], bf16)
            bias_t = wpool.tile([128, 1], f32)
            dummy = wpool.tile([128, 512], bf16)
            xts = [xpool.tile([128, CUBE], bf16, name=f"xt{k}")
                   for k in range(5)]
            xss = [xpool.tile([128, CUBE], bf16, name=f"xs{k}")
                   for k in range(5)]

            # small PE warm-up: start the HAM clock ramp while DMAs land
            nc.any.memset(dummy, 0)
            warm = pspool.tile([128, 512], f32, name="bank")
            for i in range(NWARM):
                nc.tensor.matmul(warm[:], dummy[:, 0:128], dummy[:],
                                 start=(i == 0), stop=(i == NWARM - 1))

            # piece layout: fine plane-pairs for the startup-critical
            # tiles (k=0,1), coarse for the rest
            PIECES = [(0, 2), (2, 4), (4, 11), (11, 18)]
            CPIECES = [(0, 4), (4, 11), (11, 18)]

            def wdma(q, wt_t, w_d, lo_t, hi_t):
                q.dma_start(wt_t[:, lo_t * 128:hi_t * 128],
                            w_d[:, lo_t * 128:hi_t * 128])

            def xdma(q, t, d, k, p, pieces=PIECES):
                lo, hi = pieces[p]
                q.dma_start(t[k][:, lo * DSTR:hi * DSTR],
                            d[k][:, lo * DSTR:hi * DSTR])

            # Three-lane supply scheduled to chunk 0's consumption order
            # (taps kd-major, 2 matmuls per tap, ~0.25us each early):
            #   sync (hwdge):   wl taps 0-8, xt0/xss0 plane pieces, wl 21-26
            #   scalar (hwdge): wh taps 0-8, xt1/xss1 plane pieces, wh 21-26
            #   gpsimd (sw, ~4us/piece): bias + both roles of taps 9-20,
            #     which are not needed until ~18-25us into the run
            nc.gpsimd.dma_start(bias_t[:], bias_d)
            wdma(nc.gpsimd, wlt, wl_d, 9, 14)
            wdma(nc.gpsimd, wht, wh_d, 9, 14)
            wdma(nc.gpsimd, wlt, wl_d, 14, 21)
            wdma(nc.gpsimd, wht, wh_d, 14, 21)
            wdma(nc.sync, wlt, wl_d, 0, 3)
            wdma(nc.scalar, wht, wh_d, 0, 3)
            xdma(nc.sync, xts, xs_d, 0, 0)
            xdma(nc.scalar, xts, xs_d, 1, 0)
            xdma(nc.sync, xss, xw_d, 0, 0)
            xdma(nc.scalar, xss, xw_d, 1, 0)
            wdma(nc.sync, wlt, wl_d, 3, 9)
            wdma(nc.scalar, wht, wh_d, 3, 9)
            xdma(nc.sync, xts, xs_d, 0, 1)
            xdma(nc.scalar, xts, xs_d, 1, 1)
            xdma(nc.sync, xss, xw_d, 0, 1)
            xdma(nc.scalar, xss, xw_d, 1, 1)
            wdma(nc.sync, wlt, wl_d, 21, 27)
            wdma(nc.scalar, wht, wh_d, 21, 27)
            for p in (2, 3):
                xdma(nc.sync, xts, xs_d, 0, p)
                xdma(nc.scalar, xts, xs_d, 1, p)
                xdma(nc.sync, xss, xw_d, 0, p)
                xdma(nc.scalar, xss, xw_d, 1, p)
            for k in (2, 3, 4):
                for p in range(3):
                    xdma(nc.sync, xts, xs_d, k, p, CPIECES)
                    xdma(nc.scalar, xss, xw_d, k, p, CPIECES)

            xvs = [xt.rearrange("p (d h w) -> p d h w", d=SP, h=SP, w=SP)
                   for xt in xts]
            xsv = [xt.rearrange("p (d h w) -> p d h w", d=SP, h=SP, w=SP)
                   for xt in xss]

            TAPS = [(kd, kh, kw)
                    for kd in range(KW) for kh in range(KW) for kw in range(KW)]

            for u in range(TSH // 2):  # output pair
                stage = spool.tile([128, S * S * S], f32, name="stage")
                for c in range(8):  # 2 output d-planes per chunk
                    bank = pspool.tile([128, 512], f32, name="bank")
                    nmm = 0
                    for kd, kh, kw in TAPS:
                        i = (kd * KW + kh) * KW + kw
                        vs = xsv if kw == 1 else xvs
                        wb = 0 if kw == 1 else kw
                        for wt, xv in ((wlt, vs[u]), (wht, vs[u + 1])):
                            rhs = xv[:, 2 * c + kd:2 * c + kd + 2,
                                     kh:kh + S, wb:wb + S]
                            nc.tensor.matmul(
                                bank[:],
                                wt[:, i * 128:(i + 1) * 128],
                                rhs,
                                start=(nmm == 0), stop=(nmm == 53),
                            )
                            nmm += 1
                    # the two time-slice writes go on separate hwdge lanes;
                    # the very last chunk is evacuated in halves so its
                    # output DMA starts as early as possible
                    halves = 2 if (u == TSH // 2 - 1 and c == 7) else 1
                    hw_ = 512 // halves
                    for h in range(halves):
                        lo = c * 512 + h * hw_
                        nc.vector.tensor_scalar_add(
                            stage[:, lo:lo + hw_],
                            bank[:, h * hw_:(h + 1) * hw_], bias_t[:]
                        )
                        nc.sync.dma_start(
                            y_d[2 * u][:, lo:lo + hw_],
                            stage[0:C, lo:lo + hw_],
                        )
                        nc.scalar.dma_start(
                            y_d[2 * u + 1][:, lo:lo + hw_],
                            stage[C:128, lo:lo + hw_],
                        )

    nc.compile()
    return nc


def _host_prep(x, weight, bias):
    """Build per-core input maps."""
    import ml_dtypes

    bf16 = ml_dtypes.bfloat16
    # padded slices: xp[b, s] = wrap-pad1 of x[b,:,s] -> (C, 18,18,18)
    xpad = np.pad(x, ((0, 0), (0, 0), (0, 0), (1, 1), (1, 1), (1, 1)),
                  mode="wrap").astype(bf16)  # (B, C, S, 18,18,18)
    # w-shifted copy: xsh[..., w] = xpad[..., w+1] (only w<17 is ever read)
    xsh = np.concatenate(
        [xpad[..., 1:], np.zeros_like(xpad[..., 0:1])], axis=-1)

    # weight block-banded lhsT tiles: [128=(j,ci), 27*128=(tap,(u,co))]
    wl = np.zeros((128, 27, 128), dtype=np.float32)
    wh = np.zeros((128, 27, 128), dtype=np.float32)
    for kd in range(KW):
        for kh in range(KW):
            for kw in range(KW):
                i = (kd * KW + kh) * KW + kw
                for j in range(2):
                    for u in range(2):
                        gl = j - u
                        if 0 <= gl < KW:
                            wl[j * C:(j + 1) * C, i, u * C:(u + 1) * C] = \
                                weight[gl, :, :, kd, kh, kw].T
                        gh = j - u + 2
                        if 0 <= gh < KW:
                            wh[j * C:(j + 1) * C, i, u * C:(u + 1) * C] = \
                                weight[gh, :, :, kd, kh, kw].T
    wl = wl.reshape(128, 27 * 128).astype(bf16)
    wh = wh.reshape(128, 27 * 128).astype(bf16)
    bias2 = np.concatenate([bias, bias]).astype(np.float32).reshape(128, 1)

    in_maps = []
    for core in range(NCORES):
        b = core // 2
        t0 = TSH * (core % 2)
        xs = np.empty((5, 128, CUBE), dtype=bf16)
        xw = np.empty((5, 128, CUBE), dtype=bf16)
        for k in range(5):
            sa = (t0 - 1 + 2 * k) % S
            sb = (t0 + 2 * k) % S
            xs[k, 0:C] = xpad[b, :, sa].reshape(C, CUBE)
            xs[k, C:128] = xpad[b, :, sb].reshape(C, CUBE)
            xw[k, 0:C] = xsh[b, :, sa].reshape(C, CUBE)
            xw[k, C:128] = xsh[b, :, sb].reshape(C, CUBE)
        in_maps.append({"xs": xs, "xw": xw, "wl": wl, "wh": wh,
                        "bias2": bias2})
    return in_maps


LAST_RESULTS = None


def kernel(x, weight, bias, _trace=False):
    global _PROGRAM, LAST_RESULTS
    from concourse import bass_utils

    x = np.asarray(x, dtype=np.float32)
    weight = np.asarray(weight, dtype=np.float32)
    bias = np.asarray(bias, dtype=np.float32)

    if _PROGRAM is None:
        _PROGRAM = _build_program()
    nc = _PROGRAM

    in_maps = _host_prep(x, weight, bias)
    res = bass_utils.run_bass_kernel_spmd(
        nc, in_maps, core_ids=list(range(NCORES)), trace=_trace
    )
    LAST_RESULTS = res

    out = np.empty((B, C, S, S, S, S), dtype=np.float32)
    for core in range(NCORES):
        b = core // 2
        t0 = TSH * (core % 2)
        y = res.results[core]["y"]  # (TSH, C, 4096)
        out[b, :, t0:t0 + TSH] = y.transpose(1, 0, 2).reshape(C, TSH, S, S, S)
    return out
